# revision 20
# baseline (speedup 1.0000x reference)
"""Segment-softmax additive-attention pooling on 8 TRN2 NeuronCores.

Math (per node n with segment b = batch_index[n]):
    beta[n]  = v . tanh(Q@W + K@U)[n]
    alpha[n] = exp(beta[n]) / sum_{m in b} exp(beta[m])
    out[b]   = sum_{n in b} alpha[n] * V[n]

Strategy:
  - batch_index is sorted -> shard rows across 8 cores at segment
    boundaries (no cross-core reduction, <=79 segments per core).
  - Host pre-transposes Q,K to [D, rows], block-transposes V to a
    [128, tiles*129] layout with a column of ones baked in after each
    128-wide V block (gives the softmax denominator for free), and
    reformats batch_index to per-tile f32 columns. Every DMA is then
    fully contiguous per partition and batched to ~2MB transfers.
  - Compute dtype fp16 (same bytes/rate as bf16, 8x the mantissa).
  - On device, per 1024-row slab (software-pipelined across slabs):
      S^T = W^T Q^T + U^T K^T        (PE, 4 matmuls, rhs free=512)
      T^T = tanh(S^T)                (ACT, one op per slab)
      beta = T @ v                   (PE, 8 matmuls of rhs free=1)
      e = exp(beta)                  (ACT)
      A[n,j] = (bi[n]==iota[j]) * e[n]   (DVE, 2 broadcast tensor_tensor)
      NumG[j,:] += A^T @ [V | 1]     (PE, accumulated in one PSUM bank)
  - Final: out[j,:] = NumG[j,0:128] / max(NumG[j,128],1e-30), DMA out.
  - Scatter-add is expressed as matmul with a one-hot-weighted A, so no
    indirect addressing at all; softmax normalization is folded into a
    single division at the end.  ~156us on HW vs ~137us HBM roofline.
"""

import numpy as np

N_CORES = 8
D = 128
NUM_SEGMENTS = 512
SLABW = 1024
NSLAB = 63
R_PAD = NSLAB * SLABW        # 64512 padded rows per core
T_TILES = R_PAD // 128       # 504 tiles of 128 rows
G_TILES = SLABW // 128       # 8 row-tiles per slab
NB = 80                      # local segment slots per core (partition dim)
PAD_SLOT = NB - 1            # local slot that padding rows are routed to

DT_NAME = "float16"          # compute dtype for Q/K/V/W/U/v ("float32"|"bfloat16")

_compiled = {}
LAST_RESULT = None


def _build_nc(dt_name, nslab=NSLAB):
    import concourse.bass as bass
    import concourse.bacc as bacc
    import concourse.tile as tile
    from concourse import mybir

    NSLAB_ = nslab
    R_PAD_ = NSLAB_ * SLABW
    T_TILES_ = R_PAD_ // 128
    # DMA load groups (in slabs): small first groups so compute starts early
    group_sizes = []
    rem = NSLAB_
    for want in [1, 1, 2]:
        if rem > 0:
            g = min(want, rem)
            group_sizes.append(g)
            rem -= g
    import os as _os
    _lg = int(_os.environ.get("K_LOADG", "6"))
    while rem > 0:
        g = min(_lg, rem)
        group_sizes.append(g)
        rem -= g
    LOADG_MAX = max(group_sizes)
    group_start = [0]
    for g in group_sizes:
        group_start.append(group_start[-1] + g)

    dt = getattr(mybir.dt, dt_name)
    f32 = mybir.dt.float32
    nc = bacc.Bacc("TRN2", target_bir_lowering=False, debug=False,
                   num_devices=N_CORES)

    qt_d = nc.dram_tensor("qt", [128, R_PAD_], dt, kind="ExternalInput").ap()
    kt_d = nc.dram_tensor("kt", [128, R_PAD_], dt, kind="ExternalInput").ap()
    vr_d = nc.dram_tensor("vr", [128, T_TILES_ * 129], dt, kind="ExternalInput").ap()
    bic_d = nc.dram_tensor("bic", [128, T_TILES_], f32, kind="ExternalInput").ap()
    iota_d = nc.dram_tensor("iota", [128, NB], f32, kind="ExternalInput").ap()
    w_d = nc.dram_tensor("w", [128, 128], dt, kind="ExternalInput").ap()
    u_d = nc.dram_tensor("u", [128, 128], dt, kind="ExternalInput").ap()
    vv_d = nc.dram_tensor("vv", [128, 1], dt, kind="ExternalInput").ap()
    out_d = nc.dram_tensor("out", [NB, 128], f32, kind="ExternalOutput").ap()

    Tanh = mybir.ActivationFunctionType.Tanh
    Exp = mybir.ActivationFunctionType.Exp
    is_equal = mybir.AluOpType.is_equal
    mult = mybir.AluOpType.mult

    _bufs = int(_os.environ.get("K_BUFS", "3"))
    with tile.TileContext(nc) as tc, \
         tc.tile_pool(name="const", bufs=1) as constp, \
         tc.tile_pool(name="qk", bufs=_bufs) as qkp, \
         tc.tile_pool(name="vsl", bufs=_bufs) as vslp, \
         tc.tile_pool(name="tt", bufs=3) as ttp, \
         tc.tile_pool(name="sm", bufs=4) as smp, \
         tc.tile_pool(name="at", bufs=3) as atp, \
         tc.tile_pool(name="fin", bufs=1) as finp, \
         tc.tile_pool(name="ps_s", bufs=2, space="PSUM") as pss, \
         tc.tile_pool(name="ps_b", bufs=2, space="PSUM") as psb, \
         tc.tile_pool(name="ps_o", bufs=1, space="PSUM") as pso:

        wt = constp.tile([128, 128], dt)
        nc.sync.dma_start(out=wt, in_=w_d)
        ut = constp.tile([128, 128], dt)
        nc.sync.dma_start(out=ut, in_=u_d)
        vv = constp.tile([128, 1], dt)
        nc.scalar.dma_start(out=vv, in_=vv_d)
        iota = constp.tile([128, NB], f32)
        nc.scalar.dma_start(out=iota, in_=iota_d)
        bic = constp.tile([128, T_TILES_], f32)
        nc.scalar.dma_start(out=bic, in_=bic_d)

        numg = pso.tile([NB, 129], f32)

        # pipeline state per slab
        vr_s = [None] * NSLAB_
        st_s = [None] * NSLAB_
        tt_s = [None] * NSLAB_
        bp_s = [None] * NSLAB_
        eb_s = [None] * NSLAB_
        at_s = [None] * NSLAB_

        qt_g = [None]
        kt_g = [None]
        vr_g = [None]
        g_base = [0]

        def stage_load_group(g):
            s0 = group_start[g]
            ns = group_sizes[g]
            w0 = s0 * SLABW
            w1 = w0 + ns * SLABW
            qt_t = qkp.tile([128, LOADG_MAX * SLABW], dt, tag="qt")
            nc.sync.dma_start(out=qt_t[:, :w1 - w0], in_=qt_d[:, w0:w1])
            kt_t = qkp.tile([128, LOADG_MAX * SLABW], dt, tag="kt")
            nc.sync.dma_start(out=kt_t[:, :w1 - w0], in_=kt_d[:, w0:w1])
            t0 = s0 * G_TILES
            t1 = t0 + ns * G_TILES
            vr_t = vslp.tile([128, LOADG_MAX * G_TILES, 129], dt, tag="vr")
            nc.sync.dma_start(
                out=vr_t[:, :t1 - t0, :],
                in_=vr_d[:, t0 * 129:t1 * 129].rearrange(
                    "p (t d) -> p t d", d=129))
            qt_g[0], kt_g[0], vr_g[0] = qt_t, kt_t, vr_t
            g_base[0] = s0

        next_group = [0]

        def stage_load(s):
            if next_group[0] < len(group_start) - 1 and s == group_start[next_group[0]]:
                stage_load_group(next_group[0])
                next_group[0] += 1
            o = (s - g_base[0]) * SLABW
            vr_s[s] = vr_g[0][:, (s - g_base[0]) * G_TILES:
                              (s - g_base[0] + 1) * G_TILES, :]
            return (qt_g[0][:, o:o + SLABW], kt_g[0][:, o:o + SLABW])

        def stage_s(s, qt_t, kt_t):
            st = pss.tile([128, SLABW], f32, tag="st")
            for h in range(SLABW // 512):
                sl = slice(h * 512, (h + 1) * 512)
                nc.tensor.matmul(st[:, sl], lhsT=wt, rhs=qt_t[:, sl],
                                 start=True, stop=False)
                nc.tensor.matmul(st[:, sl], lhsT=ut, rhs=kt_t[:, sl],
                                 start=False, stop=True)
            st_s[s] = st

        def stage_tanh(s):
            tt = ttp.tile([128, SLABW], dt, tag="tt")
            nc.scalar.activation(out=tt, in_=st_s[s], func=Tanh)
            tt_s[s] = tt
            st_s[s] = None

        def stage_beta(s):
            bp = psb.tile([128, G_TILES], f32, tag="bp")
            tt = tt_s[s]
            for t in range(G_TILES):
                nc.tensor.matmul(bp[:, t:t + 1],
                                 lhsT=tt[:, t * 128:(t + 1) * 128],
                                 rhs=vv, start=True, stop=True)
            bp_s[s] = bp

        def stage_exp(s):
            eb = smp.tile([128, G_TILES], dt, tag="eb")
            nc.scalar.activation(out=eb, in_=bp_s[s], func=Exp)
            eb_s[s] = eb
            bp_s[s] = None
            tt_s[s] = None

        def stage_a(s):
            at = atp.tile([128, G_TILES, NB], dt, tag="at")
            bic_b = bic[:, s * G_TILES:(s + 1) * G_TILES].broadcast_to(
                (128, G_TILES, NB))
            iota_b = bass.AP(
                tensor=iota.tensor, offset=iota.offset,
                ap=[iota.ap[0], [0, G_TILES], iota.ap[1]])
            nc.vector.tensor_tensor(out=at, in0=bic_b, in1=iota_b,
                                    op=is_equal)
            eb_b = eb_s[s].broadcast_to((128, G_TILES, NB))
            nc.vector.tensor_tensor(out=at, in0=at, in1=eb_b, op=mult)
            at_s[s] = at
            eb_s[s] = None

        def stage_pool(s):
            for t in range(G_TILES):
                g = s * G_TILES + t
                nc.tensor.matmul(numg, lhsT=at_s[s][:, t, :],
                                 rhs=vr_s[s][:, t, :],
                                 start=(g == 0),
                                 stop=(g == NSLAB_ * G_TILES - 1),
                                 skip_group_check=True)
            at_s[s] = None
            vr_s[s] = None

        for i in range(NSLAB_ + 2):
            if i < NSLAB_:
                qt_t, kt_t = stage_load(i)
                stage_s(i, qt_t, kt_t)
                stage_tanh(i)
            j = i - 1
            if 0 <= j < NSLAB_:
                stage_beta(j)
                stage_exp(j)
                stage_a(j)
            k = i - 2
            if 0 <= k < NSLAB_:
                stage_pool(k)

        gc = finp.tile([NB, 1], f32)
        nc.vector.tensor_scalar(out=gc, in0=numg[:, 128:129],
                                scalar1=1e-30, scalar2=None,
                                op0=mybir.AluOpType.max)
        recip = finp.tile([NB, 1], f32)
        nc.vector.reciprocal(out=recip, in_=gc)
        outt = finp.tile([NB, 128], f32)
        nc.vector.tensor_scalar(out=outt, in0=numg[:, 0:128],
                                scalar1=recip, scalar2=None, op0=mult)
        nc.sync.dma_start(out=out_d, in_=outt)

    nc.compile()
    return nc


def _np_dt(dt_name):
    if dt_name == "float32":
        return np.float32
    from concourse import mybir
    return mybir.dt.np(getattr(mybir.dt, dt_name))


def _shard(Q, K, V, bi):
    """Split rows at segment boundaries into N_CORES contiguous shards."""
    N = Q.shape[0]
    seg_starts = np.searchsorted(bi, np.arange(NUM_SEGMENTS + 1)).astype(np.int64)
    split_rows = [0]
    split_segs = [0]
    for m in range(1, N_CORES):
        ideal = (N * m) // N_CORES
        s = int(np.argmin(np.abs(seg_starts - ideal)))
        s = min(max(s, split_segs[-1]), NUM_SEGMENTS)
        split_segs.append(s)
        split_rows.append(int(seg_starts[s]))
    split_segs.append(NUM_SEGMENTS)
    split_rows.append(N)
    return split_rows, split_segs


def kernel(Q, K, V, batch_index, W, U, v):
    from concourse.bass_utils import run_bass_kernel_spmd

    Q = np.asarray(Q, dtype=np.float32)
    K = np.asarray(K, dtype=np.float32)
    V = np.asarray(V, dtype=np.float32)
    W = np.asarray(W, dtype=np.float32)
    U = np.asarray(U, dtype=np.float32)
    v = np.asarray(v, dtype=np.float32)
    bi = np.asarray(batch_index).astype(np.int64)
    N = Q.shape[0]

    ndt = _np_dt(DT_NAME)
    split_rows, split_segs = _shard(Q, K, V, bi)
    max_rm = max(split_rows[m + 1] - split_rows[m] for m in range(N_CORES))
    nslab = max(1, -(-max_rm // SLABW))  # ceil
    R_PAD_ = nslab * SLABW
    T_TILES_ = R_PAD_ // 128

    iota = np.tile(np.arange(NB, dtype=np.float32)[None, :], (128, 1))
    Wc = W.astype(ndt)
    Uc = U.astype(ndt)
    vvec = v.reshape(128, 1).astype(ndt)

    in_maps = []
    for m in range(N_CORES):
        r0, r1 = split_rows[m], split_rows[m + 1]
        s0, s1 = split_segs[m], split_segs[m + 1]
        R_m = r1 - r0
        nb = s1 - s0
        assert R_m <= R_PAD_, f"core {m}: {R_m} rows > R_PAD {R_PAD_}"
        assert 0 < nb <= NB - 1, f"core {m}: {nb} segments"

        tfull, rem = divmod(R_m, 128)

        qt = np.zeros((128, R_PAD_), dtype=ndt)
        qt[:, :R_m] = Q[r0:r1].T
        kt = np.zeros((128, R_PAD_), dtype=ndt)
        kt[:, :R_m] = K[r0:r1].T

        vr = np.zeros((128, T_TILES_, 129), dtype=ndt)
        vr[:, :, 128] = 1.0
        Vp = V[r0:r1]
        vr[:, :tfull, :128] = Vp[:tfull * 128].reshape(tfull, 128, 128).transpose(1, 0, 2)
        if rem:
            vr[:rem, tfull, :128] = Vp[tfull * 128:]
        vr = vr.reshape(128, T_TILES_ * 129)

        bic = np.full((128, T_TILES_), float(PAD_SLOT), dtype=np.float32)
        bl = (bi[r0:r1] - s0).astype(np.float32)
        bic[:, :tfull] = bl[:tfull * 128].reshape(tfull, 128).T
        if rem:
            bic[:rem, tfull] = bl[tfull * 128:]

        in_maps.append({
            "qt": qt, "kt": kt, "vr": vr, "bic": bic,
            "iota": iota, "w": Wc, "u": Uc, "vv": vvec,
        })

    import os as _os
    key = (DT_NAME, nslab, _os.environ.get('K_LOADG', '6'), _os.environ.get('K_BUFS', '3'))
    if key not in _compiled:
        _compiled[key] = _build_nc(DT_NAME, nslab=nslab)
    nc = _compiled[key]

    try:
        res = run_bass_kernel_spmd(nc, in_maps, core_ids=list(range(N_CORES)))
    except Exception:
        res = run_bass_kernel_spmd(nc, in_maps, core_ids=list(range(N_CORES)))
    global LAST_RESULT
    LAST_RESULT = res

    out = np.zeros((NUM_SEGMENTS, D), dtype=np.float32)
    for m in range(N_CORES):
        s0, s1 = split_segs[m], split_segs[m + 1]
        out[s0:s1] = res.results[m]["out"][:s1 - s0]
    return out
